# revision 3
# baseline (speedup 1.0000x reference)
"""2x2/stride-2 NHWC max pool on (32,112,112,128) f32, data-parallel over 8 NeuronCores.

Sharding: batch dim 32 -> 4 images per core (pure data parallel, no communication).
Per core, each pair of images maps (b in 2, out_row in 56) -> 112 SBUF partitions;
a W-chunk of the two input rows feeding each output row lands in that row's
partition, so the 2x2 window reduces to two DVE tensor_max ops per tile:
  1. vertical:   max(row 2i, row 2i+1), written in place  (contiguous, unit stride)
  2. horizontal: max(adjacent 128-channel blocks)         (stride 2*128 between blocks)
Input DMAs issue from the sync queue, output DMAs from the scalar queue so the
two HWDGE rings load/store concurrently. The kernel is DMA-bound: 25.7 MB read
+ 6.4 MB written per core ~= 74 us at the 436 GB/s SBUF-AXI fabric ceiling.
"""

import sys

sys.path.insert(0, "/opt/trn_rl_repo")

import numpy as np

import concourse.bass as bass
import concourse.tile as tile
from concourse import bacc, mybir
from concourse.bass_utils import run_bass_kernel_spmd

N_CORES = 8
B, H, W, C = 32, 112, 112, 128
BPC = B // N_CORES  # batches per core
HO, WO = H // 2, W // 2
WC = 28  # input w-positions per chunk
NW = W // WC
JC = WC // 2  # output w-positions per chunk

_cache: dict = {}


def _build(reps: int = 1):
    nc = bacc.Bacc("TRN2", target_bir_lowering=False, debug=False, num_devices=N_CORES)
    a = nc.dram_tensor("a", [BPC, H, W, C], mybir.dt.float32, kind="ExternalInput").ap()
    o = nc.dram_tensor(
        "out", [BPC, HO, WO, C], mybir.dt.float32, kind="ExternalOutput"
    ).ap()

    with tile.TileContext(nc) as tc:
        with tc.tile_pool(name="pool", bufs=5) as pool:
            for _ in range(reps):
                for bp in range(BPC // 2):
                    for w in range(NW):
                        tin = pool.tile([2 * HO, 2, WC * C], mybir.dt.float32, tag="tin")
                        src = a[2 * bp : 2 * bp + 2, :, WC * w : WC * (w + 1), :].rearrange(
                            "b (i r) w c -> (b i) r (w c)", r=2
                        )
                        nc.sync.dma_start(out=tin[:], in_=src)

                        tv = tin[:, 0, :]
                        nc.vector.tensor_max(out=tv, in0=tin[:, 0, :], in1=tin[:, 1, :])

                        to = pool.tile([2 * HO, JC * C], mybir.dt.float32, tag="to")
                        tvv = tv.rearrange("p (j s c) -> p j s c", s=2, c=C)
                        nc.vector.tensor_max(
                            out=to[:].rearrange("p (j c) -> p j c", c=C),
                            in0=tvv[:, :, 0, :],
                            in1=tvv[:, :, 1, :],
                        )

                        dst = o[2 * bp : 2 * bp + 2, :, JC * w : JC * (w + 1), :].rearrange(
                            "b i j c -> (b i) (j c)"
                        )
                        nc.scalar.dma_start(out=dst, in_=to[:])

    nc.compile()
    return nc


def _get_nc():
    if "nc" not in _cache:
        _cache["nc"] = _build()
    return _cache["nc"]


def kernel(a: np.ndarray) -> np.ndarray:
    nc = _get_nc()
    in_maps = [
        {"a": np.ascontiguousarray(a[i * BPC : (i + 1) * BPC])} for i in range(N_CORES)
    ]
    res = run_bass_kernel_spmd(nc, in_maps, list(range(N_CORES))).results
    return np.concatenate([res[i]["out"] for i in range(N_CORES)], axis=0)


# revision 5
# speedup vs baseline: 1.2715x; 1.2715x over previous
"""2x2/stride-2 NHWC max pool on (32,112,112,128) f32, data-parallel over 8 NeuronCores.

Sharding: batch dim 32 -> 4 images per core (pure data parallel, no communication).
Per core, each pair of images maps (b in 2, out_row in 56) -> 112 SBUF partitions;
a W-chunk of the two input rows feeding each output row lands in that row's
partition, so the 2x2 window reduces to two DVE tensor_max ops per tile:
  1. vertical:   max(row 2i, row 2i+1)           (contiguous, unit stride)
  2. horizontal: max(adjacent 128-channel blocks) (stride 2*128 between blocks)
The kernel is HBM-bound: 25.7 MB read + 6.4 MB written per core; with all 8
cores active the chip HBM sustains ~270 GB/s/core, ~115 us/pass steady state.
"""

import sys

sys.path.insert(0, "/opt/trn_rl_repo")

import numpy as np

import concourse.bass as bass
import concourse.tile as tile
from concourse import bacc, mybir
from concourse.bass_utils import run_bass_kernel_spmd

N_CORES = 8
B, H, W, C = 32, 112, 112, 128
BPC = B // N_CORES  # batches per core
HO, WO = H // 2, W // 2
WC = 28  # input w-positions per chunk
NW = W // WC
JC = WC // 2  # output w-positions per chunk

_cache: dict = {}


def _build(reps: int = 1):
    nc = bacc.Bacc("TRN2", target_bir_lowering=False, debug=False, num_devices=N_CORES)
    a = nc.dram_tensor("a", [BPC, H, W, C], mybir.dt.float32, kind="ExternalInput").ap()
    o = nc.dram_tensor(
        "out", [BPC, HO, WO, C], mybir.dt.float32, kind="ExternalOutput"
    ).ap()

    with tile.TileContext(nc) as tc:
        with tc.tile_pool(name="pool", bufs=3) as pool:
            for _ in range(reps):
                for bp in range(BPC // 2):
                    for w in range(NW):
                        tin = pool.tile([2 * HO, 2, WC * C], mybir.dt.float32, tag="tin")
                        src = a[2 * bp : 2 * bp + 2, :, WC * w : WC * (w + 1), :].rearrange(
                            "b (i r) w c -> (b i) r (w c)", r=2
                        )
                        nc.sync.dma_start(out=tin[:], in_=src)

                        tv = pool.tile([2 * HO, WC * C], mybir.dt.float32, tag="tv")
                        nc.vector.tensor_max(
                            out=tv[:], in0=tin[:, 0, :], in1=tin[:, 1, :]
                        )

                        to = pool.tile([2 * HO, JC * C], mybir.dt.float32, tag="to")
                        tvv = tv[:].rearrange("p (j s c) -> p j s c", s=2, c=C)
                        nc.vector.tensor_max(
                            out=to[:].rearrange("p (j c) -> p j c", c=C),
                            in0=tvv[:, :, 0, :],
                            in1=tvv[:, :, 1, :],
                        )

                        dst = o[2 * bp : 2 * bp + 2, :, JC * w : JC * (w + 1), :].rearrange(
                            "b i j c -> (b i) (j c)"
                        )
                        nc.sync.dma_start(out=dst, in_=to[:])

    nc.compile()
    return nc


def _get_nc():
    if "nc" not in _cache:
        _cache["nc"] = _build()
    return _cache["nc"]


def kernel(a: np.ndarray) -> np.ndarray:
    nc = _get_nc()
    in_maps = [
        {"a": np.ascontiguousarray(a[i * BPC : (i + 1) * BPC])} for i in range(N_CORES)
    ]
    res = run_bass_kernel_spmd(nc, in_maps, list(range(N_CORES))).results
    return np.concatenate([res[i]["out"] for i in range(N_CORES)], axis=0)
